# revision 23
# baseline (speedup 1.0000x reference)
"""GatedDeltaNet Trainium2 kernel — 8 NeuronCores, tensor-parallel over heads.

Shapes: B=2, S=4096, D=2048, HK=HV=16, DK=DV=128, KCONV=4.

Per-core: 2 heads. Device pipeline (single NEFF, one dispatch):
  1. AllGather host-transposed bf16 hidden shards -> X^T [D, 8192] on every core.
  2. Projections X @ [Wq|Wk|Wv|Wz|Wb|Wa] for the core's heads, channel-major
     PSUM eviction with fused causal-conv(4 taps) + SiLU epilogue (qkv),
     SiLU*norm_w (z), raw rows (b/a logits).
  3. Chunked gated delta rule (chunk C=128, WY representation):
     (I+M)^-1 via nilpotent squaring (7 factors), per-chunk state update,
     outputs, gated RMSNorm.  All q/k l2-norms folded into the exp matrices.
  4. AllGather per-head core features, column-sharded out_proj.
Host: concat per-core output column slices.
"""

import os
import sys
import types

sys.path.insert(0, "/opt/trn_rl_repo")

import numpy as np

# ---------------------------------------------------------------- constants
B, S, D = 2, 4096, 2048
HK = HV = 16
DK = DV = 128
KCONV = 4
KEY_DIM = HK * DK            # 2048
VALUE_DIM = HV * DV          # 2048
CONV_DIM = 2 * KEY_DIM + VALUE_DIM
EPS = 1e-6
T = B * S                    # 8192 tokens
C = 128                      # recurrence chunk length
NCHUNK = S // C              # 32 chunks per (batch, head)
HL = 2                       # heads per core
NPAIR = HL * B               # (head, batch) pairs per core
KC = D // 128                # 16 contraction chunks
MCOL = 9                     # 128-col groups: q0 q1 k0 k1 v0 v1 z0 z1 ba
PROJ_COLS = MCOL * 128       # 1152
TT_W = 512                   # token tile width in projection/out_proj
NTT = T // TT_W              # 16
NEG = -30000.0               # additive mask "minus infinity"
HALF_LN_DK = 0.5 * float(np.log(DK))

_CACHE = {}
LAST_RESULT = None
DEBUG_TAPS = False           # dev: expose qkvT/zT/coreT as outputs


def _install_ntff_hook():
    """Make trace=True work under axon when the image lacks antenv.axon_hooks."""
    try:
        import antenv.axon_hooks  # noqa: F401
        return
    except ImportError:
        pass
    try:
        from trn_agent_boot.trn_boot import _ntff_profile_via_ctypes
        hook = _ntff_profile_via_ctypes("/opt/axon/libaxon_pjrt.so")
        if hook is None:
            return
        import antenv
        mod = types.ModuleType("antenv.axon_hooks")
        mod.get_axon_ntff_profile_hook = lambda: hook
        mod.set_axon_ntff_profile_hook = lambda h: None
        sys.modules["antenv.axon_hooks"] = mod
        antenv.axon_hooks = mod
    except Exception:
        pass


# ---------------------------------------------------------------- device IR
def build_module(world=8, tok_shard=None):
    import concourse.bacc as bacc
    import concourse.mybir as mybir
    import concourse.tile as tile
    from concourse.masks import make_identity

    fp32 = mybir.dt.float32
    bf16 = mybir.dt.bfloat16
    AF = mybir.ActivationFunctionType
    ALU = mybir.AluOpType

    if tok_shard is None:
        tok_shard = T // world

    nc = bacc.Bacc(None, target_bir_lowering=False)

    # ---- I/O
    xT = nc.declare_dram_parameter("xT", [D, tok_shard], bf16, isOutput=False)
    wproj = nc.declare_dram_parameter("wproj", [D, PROJ_COLS], bf16, isOutput=False)
    convw = nc.declare_dram_parameter("convw", [6 * 128, KCONV], fp32, isOutput=False)
    dtb_col = nc.declare_dram_parameter("dtb_col", [128, 1], fp32, isOutput=False)
    negA_col = nc.declare_dram_parameter("negA_col", [128, 1], fp32, isOutput=False)
    normw_col = nc.declare_dram_parameter("normw_col", [128, 1], fp32, isOutput=False)
    wout = nc.declare_dram_parameter("wout", [world * 256, 256], bf16, isOutput=False)
    out_colsT = nc.declare_dram_parameter("out_colsT", [256, T], fp32, isOutput=True)

    # ---- internal DRAM
    HD = D // 2
    ag_in0 = nc.dram_tensor("ag_in0", [HD, tok_shard], bf16)
    ag_in1 = nc.dram_tensor("ag_in1", [HD, tok_shard], bf16)
    xT_h0 = nc.dram_tensor("xT_h0", [world * HD, tok_shard], bf16,
                           addr_space="Shared")
    xT_h1 = nc.dram_tensor("xT_h1", [world * HD, tok_shard], bf16,
                           addr_space="Shared")
    qkvT = nc.dram_tensor("qkvT", [6 * 128, T], bf16)    # q0 q1 k0 k1 v0 v1
    zT = nc.dram_tensor("zT", [2 * 128, T], bf16)        # z0 z1 (silu * norm_w)
    rows_dram = nc.dram_tensor("rows_dram", [8, T], fp32)  # b0 b1 a0 a1 sq0 sq1 sk0 sk1
    scal_dram = nc.dram_tensor("scal_dram", [9, 128, 128], fp32)
    O_dram = nc.dram_tensor("O_dram", [T, 2 * 128], bf16)   # pre-gate core, token-major
    rms_dram = nc.dram_tensor("rms_dram", [128, 128], fp32)  # [rp, t]
    NSEG = 4
    TSEG = T // NSEG
    coreT_s = [nc.dram_tensor(f"coreT{s}", [2 * 128, TSEG], bf16)
               for s in range(NSEG)]
    cT_all_s = [nc.dram_tensor(f"cT_all{s}", [world * 256, TSEG], bf16,
                               addr_space="Shared")
                for s in range(NSEG)]

    if DEBUG_TAPS:
        dbg_qkvT = nc.declare_dram_parameter("dbg_qkvT", [6 * 128, T], fp32, isOutput=True)
        dbg_zT = nc.declare_dram_parameter("dbg_zT", [2 * 128, T], fp32, isOutput=True)
        dbg_rows = nc.declare_dram_parameter("dbg_rows", [8, T], fp32, isOutput=True)
        dbg_coreT = nc.declare_dram_parameter("dbg_coreT", [2 * 128, T], fp32, isOutput=True)
        dbg_chunk = nc.declare_dram_parameter("dbg_chunk", [10, 128, 128], fp32, isOutput=True)

    groups = [list(range(world))]

    from contextlib import ExitStack
    with tile.TileContext(nc) as tc, ExitStack() as stack:
        # ============================================================ consts
        const = stack.enter_context(tc.tile_pool(name="const", bufs=1))
        ident_bf = const.tile([128, 128], bf16)
        make_identity(nc, ident_bf)
        ones128_bf = const.tile([128, 1], bf16)
        nc.vector.memset(ones128_bf[:], 1.0)
        ones1_f = const.tile([1, 128], fp32)
        nc.vector.memset(ones1_f[:], 1.0)
        eye_bf = const.tile([128, 128], bf16)
        make_identity(nc, eye_bf)
        maskS = const.tile([128, 128], fp32)   # 0 where j<t else NEG
        nc.gpsimd.memset(maskS[:], NEG)
        nc.gpsimd.affine_select(out=maskS[:], in_=maskS[:],
                                compare_op=ALU.is_ge, fill=0.0, base=0,
                                pattern=[[-1, 128]], channel_multiplier=1)
        maskI = const.tile([128, 128], fp32)   # -0.5*ln(DK) where i<=t else NEG
        nc.gpsimd.memset(maskI[:], NEG)
        nc.gpsimd.affine_select(out=maskI[:], in_=maskI[:],
                                compare_op=ALU.is_gt, fill=-HALF_LN_DK, base=0,
                                pattern=[[-1, 128]], channel_multiplier=1)
        # 6*128 channels -> [128, 6, KCONV]
        convw_sb = const.tile([128, 6, KCONV], fp32)
        nc.sync.dma_start(out=convw_sb[:],
                          in_=convw.ap().rearrange("(g p) k -> p g k", p=128))
        dtb_sb = const.tile([128, 1], fp32)
        nc.sync.dma_start(out=dtb_sb[:], in_=dtb_col[:])
        negA_sb = const.tile([128, 1], fp32)
        nc.sync.dma_start(out=negA_sb[:], in_=negA_col[:])
        normw_sb = const.tile([128, 1], fp32)
        nc.sync.dma_start(out=normw_sb[:], in_=normw_col[:])
        eps_col = const.tile([128, 1], fp32)
        nc.vector.memset(eps_col[:], EPS)
        nhldk_col = const.tile([128, 1], fp32)
        nc.vector.memset(nhldk_col[:], -HALF_LN_DK)

        # ============================================================ phase 1
        # AllGather X^T in two D-halves so first-half matmuls overlap AG #2
        nc.sync.dma_start(out=ag_in0[:], in_=xT[0:HD, :])
        nc.sync.dma_start(out=ag_in1[:], in_=xT[HD:D, :])
        nc.gpsimd.collective_compute(
            "AllGather", ALU.bypass, replica_groups=groups,
            ins=[ag_in0.ap().opt()], outs=[xT_h0.ap().opt()])
        nc.gpsimd.collective_compute(
            "AllGather", ALU.bypass, replica_groups=groups,
            ins=[ag_in1.ap().opt()], outs=[xT_h1.ap().opt()])

        KCH = KC // 2
        xT_view0 = xT_h0.ap().rearrange("(w kc p) t -> w kc p t", w=world, p=128)
        xT_view1 = xT_h1.ap().rearrange("(w kc p) t -> w kc p t", w=world, p=128)

        with (
            tc.tile_pool(name="pw", bufs=1) as pw,
            tc.tile_pool(name="px", bufs=2) as px,
            tc.tile_pool(name="pev", bufs=2) as pev,
            tc.tile_pool(name="phal", bufs=1) as phal,
            tc.tile_pool(name="pps", bufs=1, space="PSUM") as pps,
            tc.tile_pool(name="pss", bufs=2, space="PSUM") as pss,
        ):
            wsb = pw.tile([128, KC, PROJ_COLS], bf16)
            nc.sync.dma_start(out=wsb[:],
                              in_=wproj.ap().rearrange("(kc p) m -> p kc m", p=128))

            halos = [phal.tile([128, KCONV - 1], fp32, name=f"halo{g}") for g in range(6)]
            # rows collected on-chip: b0 b1 a0 a1 | ssq q0 q1 k0 k1


            GW = 4 * TT_W                # token-group width (4 psum banks)
            for ttg in range(NTT // 4):
                xtile = px.tile([128, KC, GW], bf16, tag="xt", name="xt")
                for i in range(4):
                    tt = ttg * 4 + i
                    g0 = tt * TT_W
                    blk = g0 // tok_shard
                    off = g0 % tok_shard
                    nc.sync.dma_start(out=xtile[:, 0:KCH, i * TT_W:(i + 1) * TT_W],
                                      in_=xT_view0[blk, :, :, off:off + TT_W]
                                      .transpose([1, 0, 2]))
                    nc.sync.dma_start(out=xtile[:, KCH:KC, i * TT_W:(i + 1) * TT_W],
                                      in_=xT_view1[blk, :, :, off:off + TT_W]
                                      .transpose([1, 0, 2]))
                for m in range(MCOL):
                    psums = [pps.tile([128, TT_W], fp32, tag=f"pp{i}",
                                      name=f"pp{i}") for i in range(4)]
                    for kc in range(KC):
                        for i in range(4):
                            nc.tensor.matmul(psums[i][:],
                                             wsb[:, kc, m * 128:(m + 1) * 128],
                                             xtile[:, kc, i * TT_W:(i + 1) * TT_W],
                                             start=(kc == 0), stop=(kc == KC - 1))
                    for i in range(4):
                        tt = ttg * 4 + i
                        g0 = tt * TT_W
                        psum = psums[i]
                        batch_start = (g0 % S) == 0
                        if m < 6:
                            # causal conv + silu -> qkvT
                            xe = pev.tile([128, TT_W + KCONV - 1], fp32, tag="xe")
                            if batch_start:
                                nc.vector.memset(xe[:, 0:KCONV - 1], 0.0)
                            else:
                                nc.vector.tensor_copy(xe[:, 0:KCONV - 1], halos[m][:])
                            nc.scalar.copy(xe[:, KCONV - 1:], psum[:])
                            nc.vector.tensor_copy(halos[m][:], psum[:, TT_W - (KCONV - 1):])
                            wcol = convw_sb[:, m, :]
                            y0 = pev.tile([128, TT_W], fp32, tag="y0")
                            y1 = pev.tile([128, TT_W], fp32, tag="y1")
                            nc.vector.tensor_scalar_mul(y0[:], xe[:, 3:3 + TT_W],
                                                        wcol[:, 3:4])
                            nc.vector.scalar_tensor_tensor(
                                y1[:], xe[:, 2:2 + TT_W], wcol[:, 2:3], y0[:],
                                op0=ALU.mult, op1=ALU.add)
                            nc.vector.scalar_tensor_tensor(
                                y0[:], xe[:, 1:1 + TT_W], wcol[:, 1:2], y1[:],
                                op0=ALU.mult, op1=ALU.add)
                            nc.vector.scalar_tensor_tensor(
                                y1[:], xe[:, 0:0 + TT_W], wcol[:, 0:1], y0[:],
                                op0=ALU.mult, op1=ALU.add)
                            sg = pev.tile([128, TT_W], fp32, tag="sg")
                            nc.scalar.activation(sg[:], y1[:], AF.Sigmoid)
                            act = pev.tile([128, TT_W], bf16, tag="act")
                            nc.vector.tensor_tensor(act[:], y1[:], sg[:], op=ALU.mult)
                            nc.sync.dma_start(out=qkvT[m * 128:(m + 1) * 128,
                                                       g0:g0 + TT_W], in_=act[:])
                            if m < 4:
                                sq = pev.tile([128, TT_W], bf16, tag="sq")
                                nc.scalar.activation(sq[:], act[:], AF.Square)
                                ss_ps = pss.tile([1, TT_W], fp32)
                                nc.tensor.matmul(ss_ps[:], ones128_bf[:], sq[:],
                                                 start=True, stop=True)
                                ssst = pev.tile([1, TT_W], fp32, tag="ssst")
                                nc.vector.tensor_copy(ssst[:], ss_ps[:])
                                nc.sync.dma_start(
                                    out=rows_dram[4 + m:5 + m, g0:g0 + TT_W],
                                    in_=ssst[:])
                        elif m < 8:
                            # z: silu * norm_w
                            zs = pev.tile([128, TT_W], fp32, tag="zs")
                            nc.scalar.activation(zs[:], psum[:], AF.Sigmoid)
                            zb = pev.tile([128, TT_W], bf16, tag="zb")
                            nc.vector.scalar_tensor_tensor(
                                zb[:], psum[:], normw_sb[:], zs[:],
                                op0=ALU.mult, op1=ALU.mult)
                            nc.sync.dma_start(out=zT[(m - 6) * 128:(m - 5) * 128,
                                                     g0:g0 + TT_W], in_=zb[:])
                        else:
                            bast = pev.tile([4, TT_W], fp32, tag="bast")
                            nc.vector.tensor_copy(bast[:], psum[0:4, :])
                            nc.sync.dma_start(out=rows_dram[0:4, g0:g0 + TT_W],
                                              in_=bast[:])



        if DEBUG_TAPS:
            with tc.tile_pool(name="dbg", bufs=2) as pd:
                for g in range(6):
                    for j in range(4):
                        tdb = pd.tile([128, 2048], bf16, tag="tdb")
                        nc.sync.dma_start(out=tdb[:], in_=qkvT[g * 128:(g + 1) * 128, j * 2048:(j + 1) * 2048])
                        tdf = pd.tile([128, 2048], fp32, tag="tdf")
                        nc.vector.tensor_copy(tdf[:], tdb[:])
                        nc.sync.dma_start(out=dbg_qkvT[g * 128:(g + 1) * 128, j * 2048:(j + 1) * 2048], in_=tdf[:])
                for g in range(2):
                    for j in range(4):
                        tdb = pd.tile([128, 2048], bf16, tag="tdb")
                        nc.sync.dma_start(out=tdb[:], in_=zT[g * 128:(g + 1) * 128, j * 2048:(j + 1) * 2048])
                        tdf = pd.tile([128, 2048], fp32, tag="tdf")
                        nc.vector.tensor_copy(tdf[:], tdb[:])
                        nc.sync.dma_start(out=dbg_zT[g * 128:(g + 1) * 128, j * 2048:(j + 1) * 2048], in_=tdf[:])
                for j in range(8):
                    tdf = pd.tile([8, 1024], fp32, tag="tdr")
                    nc.sync.dma_start(out=tdf[:], in_=rows_dram[:, j * 1024:(j + 1) * 1024])
                    nc.sync.dma_start(out=dbg_rows[:, j * 1024:(j + 1) * 1024], in_=tdf[:])

        # ====================================================== phase 1.5
        # Per-token scalar machinery. Big tiles [128 = (lh,b,chunk), 128 = t].
        with (
            tc.tile_pool(name="pscal", bufs=1) as ps,
            tc.tile_pool(name="pscal2", bufs=2) as ps2,
        ):
            def big_from_rows(row0, name):
                t_ = ps.tile([128, 128], fp32, name=name)
                nc.sync.dma_start(
                    out=t_[:],
                    in_=rows_dram.ap()[row0:row0 + 2, :]
                    .rearrange("lh (b n t) -> (lh b n) t", b=B, n=NCHUNK, t=C))
                return t_

            blog_big = big_from_rows(0, "blog")
            alog_big = big_from_rows(2, "alog")
            ssqq_big = big_from_rows(4, "ssqq")
            ssqk_big = big_from_rows(6, "ssqk")

            zeros_big = ps.tile([128, 128], fp32, name="zeros")
            nc.vector.memset(zeros_big[:], 0.0)

            # softplus(x) = relu(x) + ln(1 + exp(-|x|)), x = alog + dtb
            xsp = ps.tile([128, 128], fp32, name="xsp")
            nc.vector.tensor_scalar_add(xsp[:], alog_big[:], dtb_sb[:])
            t_abs = ps.tile([128, 128], fp32, name="t_abs")
            nc.scalar.activation(t_abs[:], xsp[:], AF.Abs)
            nc.scalar.activation(t_abs[:], t_abs[:], AF.Exp, scale=-1.0)
            nc.scalar.activation(t_abs[:], t_abs[:], AF.Ln, bias=1.0)
            t_rel = ps.tile([128, 128], fp32, name="t_rel")
            nc.scalar.activation(t_rel[:], xsp[:], AF.Relu)
            g_big = ps.tile([128, 128], fp32, name="g")
            nc.vector.tensor_tensor(g_big[:], t_abs[:], t_rel[:], op=ALU.add)
            nc.vector.tensor_scalar_mul(g_big[:], g_big[:], negA_sb[:])
            gam_big = ps.tile([128, 128], fp32, name="gam")
            nc.vector.tensor_tensor_scan(gam_big[:], g_big[:], zeros_big[:], 0.0,
                                         op0=ALU.add, op1=ALU.add)

            beta_big = ps.tile([128, 128], fp32, name="beta")
            nc.scalar.activation(beta_big[:], blog_big[:], AF.Exp, scale=-1.0)
            nc.vector.tensor_scalar_add(beta_big[:], beta_big[:], 1.0)
            nc.vector.reciprocal(beta_big[:], beta_big[:])
            nbeta_big = ps.tile([128, 128], fp32, name="nbeta")
            nc.scalar.mul(nbeta_big[:], beta_big[:], -1.0)

            lnq = ps2.tile([128, 128], fp32, tag="ln")
            nc.scalar.activation(lnq[:], ssqq_big[:], AF.Ln, bias=eps_col[:])
            u_q_big = ps.tile([128, 128], fp32, name="u_q")
            nc.vector.scalar_tensor_tensor(u_q_big[:], lnq[:], -0.5, gam_big[:],
                                           op0=ALU.mult, op1=ALU.add)
            qbar_big = ps.tile([128, 128], fp32, name="qbar")
            nc.scalar.activation(qbar_big[:], u_q_big[:], AF.Exp, bias=nhldk_col[:])

            lnk = ps2.tile([128, 128], fp32, tag="ln")
            nc.scalar.activation(lnk[:], ssqk_big[:], AF.Ln, bias=eps_col[:])
            u_k_big = ps.tile([128, 128], fp32, name="u_k")
            nc.vector.scalar_tensor_tensor(u_k_big[:], lnk[:], -0.5, gam_big[:],
                                           op0=ALU.mult, op1=ALU.add)
            v_k_big = ps.tile([128, 128], fp32, name="v_k")
            nc.vector.scalar_tensor_tensor(v_k_big[:], lnk[:], 0.5, gam_big[:],
                                           op0=ALU.mult, op1=ALU.add)
            s1_big = ps.tile([128, 128], fp32, name="s1")
            nc.scalar.activation(s1_big[:], u_k_big[:], AF.Exp)

            gl_col = gam_big[:, 127:128]
            nv_big = ps2.tile([128, 128], fp32, tag="nv")
            nc.scalar.mul(nv_big[:], v_k_big[:], -1.0)
            s2_big = ps.tile([128, 128], fp32, name="s2")
            nc.scalar.activation(s2_big[:], nv_big[:], AF.Exp, bias=gl_col)
            dl_col = ps.tile([128, 1], fp32, name="dl")
            nc.scalar.activation(dl_col[:], gl_col, AF.Exp)

            # column layouts via DRAM bounce: scal_dram[q] = big[rp, t]
            for q, t_ in enumerate([v_k_big, beta_big, nbeta_big, s1_big, s2_big,
                                    None, u_k_big, u_q_big, qbar_big]):
                if t_ is not None:
                    nc.sync.dma_start(out=scal_dram[q], in_=t_[:])
            nc.sync.dma_start(out=scal_dram[5][:, 0:1], in_=dl_col[:])

            cols_vk = ps.tile([128, 128], fp32, name="cvk")
            cols_beta = ps.tile([128, 128], fp32, name="cbe")
            cols_nbeta = ps.tile([128, 128], fp32, name="cnb")
            cols_s1 = ps.tile([128, 128], fp32, name="cs1")
            cols_s2 = ps.tile([128, 128], fp32, name="cs2")
            for q, t_ in enumerate([cols_vk, cols_beta, cols_nbeta, cols_s1, cols_s2]):
                nc.sync.dma_start(out=t_[:],
                                  in_=scal_dram[q].rearrange("rp t -> t rp"))
            dl_row = ps.tile([1, 128], fp32, name="dlrow")
            nc.sync.dma_start(out=dl_row[:],
                              in_=scal_dram[5][:, 0:1].rearrange("rp one -> one rp"))
            dec_bc = ps.tile([128, 128], fp32, name="dec")
            nc.gpsimd.partition_broadcast(dec_bc[:], dl_row[:])

            # ====================================================== phase 2
            # chunked gated delta rule
            with (
                tc.tile_pool(name="rin", bufs=6) as rin,
                tc.tile_pool(name="rwk", bufs=4) as rwk,
                tc.tile_pool(name="rst", bufs=1) as rst,
                tc.tile_pool(name="rgp", bufs=2, space="PSUM") as rgp,
                tc.tile_pool(name="rps", bufs=3, space="PSUM") as rps,
            ):
                ssq_cols = rst.tile([128, 128], fp32, name="ssqc")
                S_f32 = [rst.tile([128, 128], fp32, name=f"Sf{p}") for p in range(NPAIR)]
                S_bf = [rst.tile([128, 128], bf16, name=f"Sbf{p}") for p in range(NPAIR)]
                for p in range(NPAIR):
                    nc.vector.memset(S_f32[p][:], 0.0)
                    nc.vector.memset(S_bf[p][:], 0.0)

                for n in range(NCHUNK):
                    for p in range(NPAIR):
                        lh, b = p >> 1, p & 1
                        rp = 32 * p + n
                        tok0 = b * S + n * C

                        kq = rin.tile([128, 256], bf16, tag="kq")
                        kT = kq[:, 0:128]
                        qT = kq[:, 128:256]
                        vT = rin.tile([128, 128], bf16, tag="vT")
                        nc.sync.dma_start(out=kq[:, 0:128], in_=qkvT[(2 + lh) * 128:(3 + lh) * 128, tok0:tok0 + C])
                        nc.sync.dma_start(out=kq[:, 128:256], in_=qkvT[lh * 128:(lh + 1) * 128, tok0:tok0 + C])
                        nc.sync.dma_start(out=vT[:], in_=qkvT[(4 + lh) * 128:(5 + lh) * 128, tok0:tok0 + C])

                        # broadcast rows u_k,u_q,qbar via stride-0 DMA replication
                        bc_ps = rwk.tile([128, 384], fp32, tag="bc", name="bc")
                        for qi in range(3):
                            nc.sync.dma_start(
                                out=bc_ps[:, qi * 128:(qi + 1) * 128],
                                in_=scal_dram[6 + qi][rp:rp + 1, :]
                                .to_broadcast((128, 128)))

                        # [G | P] = K [K | Q]^T in one matmul (raw, [j|i, t])
                        GP_ps = rgp.tile([128, 256], fp32, tag="gp")
                        nc.tensor.matmul(GP_ps[:], kT, kq[:], start=True, stop=True)
                        G_ps = GP_ps[:, 0:128]
                        P_ps = GP_ps[:, 128:256]

                        # N = -M2^T  (strictly upper in [j,t])
                        E1 = rwk.tile([128, 128], fp32, tag="E1")
                        nc.vector.scalar_tensor_tensor(
                            E1[:], bc_ps[:, 0:128], cols_vk[:, rp:rp + 1], maskS[:],
                            op0=ALU.subtract, op1=ALU.add)
                        nc.scalar.activation(E1[:], E1[:], AF.Exp)
                        N0 = rwk.tile([128, 128], bf16, tag="N0")
                        nc.vector.scalar_tensor_tensor(
                            N0[:], G_ps, cols_nbeta[:, rp:rp + 1], E1[:],
                            op0=ALU.mult, op1=ALU.mult)

                        # squaring chain: S_j = N^(2^j), T_j = S_j^T
                        # X = Tinv2^T ~= (I+N2)(I+N) = I+N+N2+N3 (N^4 ~ 1e-4,
                        # below bf16 noise: powers of the delta-rule matrix decay
                        # fast under the gamma decay + l2-normalized keys)
                        T0_ps = rps.tile([128, 128], bf16, tag="pst")
                        nc.tensor.transpose(T0_ps[:], N0[:], ident_bf[:])
                        T0 = rwk.tile([128, 128], bf16, tag="T0", name="T0")
                        nc.scalar.copy(T0[:], T0_ps[:])
                        t1_ps = rps.tile([128, 128], fp32, tag="ps")
                        nc.tensor.matmul(t1_ps[:], N0[:], T0[:], start=True, stop=True)
                        T1 = rwk.tile([128, 128], bf16, tag="T1", name="T1")
                        nc.scalar.copy(T1[:], t1_ps[:])
                        X0 = rwk.tile([128, 128], bf16, tag="X0", name="X0")
                        nc.vector.tensor_tensor(X0[:], eye_bf[:], N0[:], op=ALU.add)
                        xp = rps.tile([128, 128], fp32, tag="ps")
                        nc.tensor.matmul(xp[:], T1[:], X0[:], start=True, stop=True)
                        X = rwk.tile([128, 128], bf16, tag="X")
                        nc.vector.tensor_tensor(X[:], X0[:], xp[:], op=ALU.add)

                        # token-major V, K variants
                        vt_ps = rps.tile([128, 128], bf16, tag="pst")
                        nc.tensor.transpose(vt_ps[:], vT[:], ident_bf[:])
                        V_tok = rwk.tile([128, 128], bf16, tag="Vtok")
                        nc.vector.tensor_copy(V_tok[:], vt_ps[:])
                        kt_ps = rps.tile([128, 128], bf16, tag="pst")
                        nc.tensor.transpose(kt_ps[:], kT[:], ident_bf[:])
                        KG = rwk.tile([128, 128], bf16, tag="KG")
                        nc.scalar.activation(KG[:], kt_ps[:], AF.Copy,
                                             scale=cols_s1[:, rp:rp + 1])
                        Kd = rwk.tile([128, 128], bf16, tag="Kd")
                        nc.vector.tensor_scalar_mul(Kd[:], kt_ps[:],
                                                    cols_s2[:, rp:rp + 1])

                        # W2^T (negated)
                        wt_ps = rps.tile([128, 128], fp32, tag="ps")
                        nc.tensor.matmul(wt_ps[:], KG[:], X[:], start=True, stop=True)
                        nWt = rwk.tile([128, 128], bf16, tag="nWt")
                        nc.scalar.mul(nWt[:], wt_ps[:], -1.0)

                        # R = beta * (U2 - W2 S)
                        R_ps = rps.tile([128, 128], fp32, tag="ps")
                        nc.tensor.matmul(R_ps[:], X[:], V_tok[:], start=True, stop=False)
                        nc.tensor.matmul(R_ps[:], nWt[:], S_bf[p][:], start=False, stop=True)
                        R = rwk.tile([128, 128], bf16, tag="R")
                        nc.scalar.activation(R[:], R_ps[:], AF.Copy,
                                             scale=cols_beta[:, rp:rp + 1])

                        # DA^T = P * exp(E) masked incl diag
                        E3 = rwk.tile([128, 128], fp32, tag="E3")
                        nc.vector.scalar_tensor_tensor(
                            E3[:], bc_ps[:, 128:256], cols_vk[:, rp:rp + 1], maskI[:],
                            op0=ALU.subtract, op1=ALU.add)
                        nc.scalar.activation(E3[:], E3[:], AF.Exp)
                        DA = rwk.tile([128, 128], bf16, tag="DA")
                        nc.vector.tensor_tensor(DA[:], P_ps, E3[:], op=ALU.mult)

                        # qbar^T = qT * bcast(qbar)
                        qbT = rwk.tile([128, 128], bf16, tag="qbT")
                        nc.vector.tensor_tensor(qbT[:], qT[:], bc_ps[:, 256:384],
                                                op=ALU.mult)

                        # O = qbar^T' S + DA' R  -> [t, dv]
                        O_ps = rps.tile([128, 128], fp32, tag="ps")
                        nc.tensor.matmul(O_ps[:], qbT[:], S_bf[p][:], start=True, stop=False)
                        nc.tensor.matmul(O_ps[:], DA[:], R[:], start=False, stop=True)

                        if DEBUG_TAPS and rp == 64:
                            for di, dt_ in enumerate([("N0", N0), ("X", X), ("KG", KG),
                                                      ("Kd", Kd), ("R", R), ("DA", DA),
                                                      ("qbT", qbT), ("Vtok", V_tok)]):
                                dnm, dtile = dt_
                                dcp = rwk.tile([128, 128], fp32, tag="dcp", name=f"dcp{di}")
                                nc.vector.tensor_copy(dcp[:], dtile[:])
                                nc.sync.dma_start(out=dbg_chunk[di], in_=dcp[:])
                            dcp8 = rwk.tile([128, 128], fp32, tag="dcp", name="dcp8")
                            nc.vector.tensor_copy(dcp8[:], O_ps[:])
                            nc.sync.dma_start(out=dbg_chunk[8], in_=dcp8[:])
                            dcp9 = rwk.tile([128, 128], fp32, tag="dcp", name="dcp9")
                            nc.vector.tensor_copy(dcp9[:], E1[:])
                            nc.sync.dma_start(out=dbg_chunk[9], in_=dcp9[:])

                        # state update: S = dec * S + Kd^T R
                        up_ps = rps.tile([128, 128], fp32, tag="ps")
                        nc.tensor.matmul(up_ps[:], Kd[:], R[:], start=True, stop=True)
                        nc.vector.scalar_tensor_tensor(
                            S_f32[p][:], S_f32[p][:], dec_bc[:, rp:rp + 1], up_ps[:],
                            op0=ALU.mult, op1=ALU.add)
                        nc.vector.tensor_copy(S_bf[p][:], S_f32[p][:])

                        # stash O (token-major) + row sum-of-squares; the
                        # rms + gating + transpose run in a later batched pass
                        sqs = rwk.tile([128, 128], fp32, tag="sqs")
                        nc.scalar.activation(sqs[:], O_ps[:], AF.Square,
                                             accum_out=ssq_cols[:, rp:rp + 1])
                        O_sb = rwk.tile([128, 128], bf16, tag="Osb")
                        nc.vector.tensor_copy(O_sb[:], O_ps[:])
                        nc.sync.dma_start(
                            out=O_dram[tok0:tok0 + C, lh * 128:(lh + 1) * 128],
                            in_=O_sb[:])

                # batched rms: ssq_cols [t, rp] -> [rp, t] -> exp(-0.5 ln(mean+eps))
                nc.sync.dma_start(out=scal_dram[5], in_=ssq_cols[:])
                ssq_rp = rst.tile([128, 128], fp32, name="ssqrp")
                nc.sync.dma_start(out=ssq_rp[:],
                                  in_=scal_dram[5].rearrange("t rp -> rp t"))
                nc.scalar.activation(ssq_rp[:], ssq_rp[:], AF.Ln, scale=1.0 / DV,
                                     bias=eps_col[:])
                nc.scalar.activation(ssq_rp[:], ssq_rp[:], AF.Exp, scale=-0.5)
                nc.sync.dma_start(out=rms_dram[:], in_=ssq_rp[:])

                # phase B: gate + transpose, segment by token range; AG per segment
                for s in range(NSEG):
                    b_s = s // 2
                    for n_s in range((s % 2) * 16, (s % 2) * 16 + 16):
                        for lh_s in range(2):
                            p_s = lh_s * 2 + b_s
                            rp_s = 32 * p_s + n_s
                            tok0s = b_s * S + n_s * C
                            Oin = rwk.tile([128, 128], bf16, tag="Oin", name="Oin")
                            nc.sync.dma_start(
                                out=Oin[:],
                                in_=O_dram[tok0s:tok0s + C,
                                           lh_s * 128:(lh_s + 1) * 128])
                            zin = rwk.tile([128, 128], bf16, tag="zin", name="zin")
                            nc.sync.dma_start(
                                out=zin[:],
                                in_=zT[lh_s * 128:(lh_s + 1) * 128,
                                       tok0s:tok0s + C])
                            rmsb = rwk.tile([128, 128], fp32, tag="rmsb",
                                            name="rmsb")
                            nc.sync.dma_start(
                                out=rmsb[:],
                                in_=rms_dram[rp_s:rp_s + 1, :]
                                .to_broadcast((128, 128)))
                            ot_ps = rps.tile([128, 128], bf16, tag="pst")
                            nc.tensor.transpose(ot_ps[:], Oin[:], ident_bf[:])
                            gr = rwk.tile([128, 128], fp32, tag="gr", name="gr")
                            nc.vector.tensor_tensor(gr[:], ot_ps[:], rmsb[:],
                                                    op=ALU.mult)
                            gct = rwk.tile([128, 128], bf16, tag="gct", name="gct")
                            nc.vector.tensor_tensor(gct[:], gr[:], zin[:],
                                                    op=ALU.mult)
                            nc.sync.dma_start(
                                out=coreT_s[s][lh_s * 128:(lh_s + 1) * 128,
                                               tok0s - s * TSEG:
                                               tok0s - s * TSEG + C],
                                in_=gct[:])
                    nc.gpsimd.collective_compute(
                        "AllGather", ALU.bypass, replica_groups=groups,
                        ins=[coreT_s[s].ap().opt()], outs=[cT_all_s[s].ap().opt()])

        if DEBUG_TAPS:
            with tc.tile_pool(name="dbg2", bufs=2) as pd:
                for g in range(2):
                    for j in range(4):
                        tdb = pd.tile([128, 2048], bf16, tag="tdb2")
                        nc.sync.dma_start(out=tdb[:], in_=coreT_s[j][g * 128:(g + 1) * 128, :])
                        tdf = pd.tile([128, 2048], fp32, tag="tdf2")
                        nc.vector.tensor_copy(tdf[:], tdb[:])
                        nc.sync.dma_start(out=dbg_coreT[g * 128:(g + 1) * 128, j * 2048:(j + 1) * 2048], in_=tdf[:])

        # ============================================================ phase 3
        # out_proj: stationary = wout half-columns, moving = gathered core,
        # output channel-major [256, T]; host transposes back.
        with (
            tc.tile_pool(name="ow", bufs=1) as ow,
            tc.tile_pool(name="oc", bufs=2) as oc,
            tc.tile_pool(name="oe", bufs=4) as oe,
            tc.tile_pool(name="ops", bufs=1, space="PSUM") as ops,
        ):
            wout_sb = ow.tile([128, 2 * world, 256], bf16)
            nc.sync.dma_start(out=wout_sb[:],
                              in_=wout.ap().rearrange("(wk p) m -> p wk m", p=128))
            for s in range(NSEG):
                cview = cT_all_s[s].ap().rearrange("(wk p) t -> wk p t", p=128)
                ctiles = []
                for i in range(4):
                    ctile = oc.tile([128, 2 * world, TT_W], bf16, tag=f"ct{i}",
                                    name=f"ct{i}")
                    nc.sync.dma_start(out=ctile[:],
                                      in_=cview[:, :, i * TT_W:(i + 1) * TT_W]
                                      .transpose([1, 0, 2]))
                    ctiles.append(ctile)
                for och in range(2):
                    psums = [ops.tile([128, TT_W], fp32, tag=f"po{i}",
                                      name=f"po{i}") for i in range(4)]
                    for wk in range(2 * world):
                        for i in range(4):
                            nc.tensor.matmul(
                                psums[i][:],
                                wout_sb[:, wk, och * 128:(och + 1) * 128],
                                ctiles[i][:, wk, :],
                                start=(wk == 0), stop=(wk == 2 * world - 1))
                    for i in range(4):
                        oev = oe.tile([128, TT_W], fp32, tag="oev", name="oev")
                        nc.scalar.copy(oev[:], psums[i][:])
                        nc.sync.dma_start(
                            out=out_colsT[och * 128:(och + 1) * 128,
                                          s * TSEG + i * TT_W:
                                          s * TSEG + (i + 1) * TT_W],
                            in_=oev[:])

    nc.finalize()
    return nc


# ---------------------------------------------------------------- host side
def _to_bf16(a):
    import ml_dtypes
    return np.asarray(a, dtype=np.float32).astype(ml_dtypes.bfloat16)


def _build_in_maps(inputs, world=8, tok_shard=None):
    if tok_shard is None:
        tok_shard = T // world
    hs = np.ascontiguousarray(np.asarray(inputs["hidden_states"],
                                         dtype=np.float32).reshape(T, D))
    W_qkv = np.asarray(inputs["W_qkv"], dtype=np.float32)
    W_z = np.asarray(inputs["W_z"], dtype=np.float32)
    W_b = np.asarray(inputs["W_b"], dtype=np.float32)
    W_a = np.asarray(inputs["W_a"], dtype=np.float32)
    conv_w = np.asarray(inputs["conv_w"], dtype=np.float32)[:, 0, :]  # [CONV_DIM, 4]
    A_log = np.asarray(inputs["A_log"], dtype=np.float32)
    dt_bias = np.asarray(inputs["dt_bias"], dtype=np.float32)
    norm_w = np.asarray(inputs["norm_w"], dtype=np.float32)
    W_out = np.asarray(inputs["W_out"], dtype=np.float32)

    in_maps = []
    for c in range(world):
        h0 = c * HL
        heads = [h0, h0 + 1]
        xT_shard = _to_bf16(hs[c * tok_shard:(c + 1) * tok_shard, :].T)

        wp = np.zeros((D, PROJ_COLS), np.float32)
        for i, h in enumerate(heads):
            wp[:, i * 128:(i + 1) * 128] = W_qkv[:, h * DK:(h + 1) * DK]
            wp[:, 256 + i * 128:256 + (i + 1) * 128] = W_qkv[:, KEY_DIM + h * DK:KEY_DIM + (h + 1) * DK]
            wp[:, 512 + i * 128:512 + (i + 1) * 128] = W_qkv[:, 2 * KEY_DIM + h * DV:2 * KEY_DIM + (h + 1) * DV]
            wp[:, 768 + i * 128:768 + (i + 1) * 128] = W_z[:, h * DV:(h + 1) * DV]
            wp[:, 1024 + i] = W_b[:, h]
            wp[:, 1026 + i] = W_a[:, h]

        cw = np.zeros((6 * 128, KCONV), np.float32)
        for i, h in enumerate(heads):
            cw[i * 128:(i + 1) * 128] = conv_w[h * DK:(h + 1) * DK]
            cw[(2 + i) * 128:(3 + i) * 128] = conv_w[KEY_DIM + h * DK:KEY_DIM + (h + 1) * DK]
            cw[(4 + i) * 128:(5 + i) * 128] = conv_w[2 * KEY_DIM + h * DV:2 * KEY_DIM + (h + 1) * DV]

        # rp = lh*64 + b*32 + n  ->  head = heads[rp // 64]
        lh_of_rp = np.arange(128) // 64
        dtb_col = dt_bias[np.array(heads)][lh_of_rp].reshape(128, 1)
        negA_col = (-np.exp(A_log))[np.array(heads)][lh_of_rp].reshape(128, 1)
        normw_col = norm_w.reshape(128, 1)
        wout_c = _to_bf16(W_out[:world * 256, c * 256:(c + 1) * 256])

        in_maps.append({
            "xT": xT_shard,
            "wproj": _to_bf16(wp),
            "convw": cw.astype(np.float32),
            "dtb_col": dtb_col.astype(np.float32),
            "negA_col": negA_col.astype(np.float32),
            "normw_col": normw_col.astype(np.float32),
            "wout": wout_c,
        })
    return in_maps


def kernel(hidden_states, W_qkv, W_z, W_b, W_a, conv_w, A_log, dt_bias,
           norm_w, W_out):
    global LAST_RESULT
    from concourse.bass_utils import run_bass_kernel_spmd

    _install_ntff_hook()
    if "nc" not in _CACHE:
        _CACHE["nc"] = build_module(world=8)
    nc = _CACHE["nc"]

    inputs = dict(hidden_states=hidden_states, W_qkv=W_qkv, W_z=W_z, W_b=W_b,
                  W_a=W_a, conv_w=conv_w, A_log=A_log, dt_bias=dt_bias,
                  norm_w=norm_w, W_out=W_out)
    in_maps = _build_in_maps(inputs, world=8)
    res = run_bass_kernel_spmd(nc, in_maps, core_ids=list(range(8)),
                               trace=bool(os.environ.get("BASS_TRACE")))
    LAST_RESULT = res
    out = np.concatenate([res.results[c]["out_colsT"].T for c in range(8)], axis=1)
    return out.reshape(B, S, D).astype(np.float32)


# revision 25
# speedup vs baseline: 1.1607x; 1.1607x over previous
"""GatedDeltaNet Trainium2 kernel — 8 NeuronCores, tensor-parallel over heads.

Shapes: B=2, S=4096, D=2048, HK=HV=16, DK=DV=128, KCONV=4.

Per-core: 2 heads. Device pipeline (single NEFF, one dispatch):
  1. AllGather host-transposed bf16 hidden shards -> X^T [D, 8192] on every core.
  2. Projections X @ [Wq|Wk|Wv|Wz|Wb|Wa] for the core's heads, channel-major
     PSUM eviction with fused causal-conv(4 taps) + SiLU epilogue (qkv),
     SiLU*norm_w (z), raw rows (b/a logits).
  3. Chunked gated delta rule (chunk C=128, WY representation):
     (I+M)^-1 via nilpotent squaring (7 factors), per-chunk state update,
     outputs, gated RMSNorm.  All q/k l2-norms folded into the exp matrices.
  4. AllGather per-head core features, column-sharded out_proj.
Host: concat per-core output column slices.
"""

import os
import sys
import types

sys.path.insert(0, "/opt/trn_rl_repo")

import numpy as np

# ---------------------------------------------------------------- constants
B, S, D = 2, 4096, 2048
HK = HV = 16
DK = DV = 128
KCONV = 4
KEY_DIM = HK * DK            # 2048
VALUE_DIM = HV * DV          # 2048
CONV_DIM = 2 * KEY_DIM + VALUE_DIM
EPS = 1e-6
T = B * S                    # 8192 tokens
C = 128                      # recurrence chunk length
NCHUNK = S // C              # 32 chunks per (batch, head)
HL = 2                       # heads per core
NPAIR = HL * B               # (head, batch) pairs per core
KC = D // 128                # 16 contraction chunks
MCOL = 9                     # 128-col groups: q0 q1 k0 k1 v0 v1 z0 z1 ba
PROJ_COLS = MCOL * 128       # 1152
TT_W = 512                   # token tile width in projection/out_proj
NTT = T // TT_W              # 16
NEG = -30000.0               # additive mask "minus infinity"
HALF_LN_DK = 0.5 * float(np.log(DK))

_CACHE = {}
LAST_RESULT = None
DEBUG_TAPS = False           # dev: expose qkvT/zT/coreT as outputs


def _install_ntff_hook():
    """Make trace=True work under axon when the image lacks antenv.axon_hooks."""
    try:
        import antenv.axon_hooks  # noqa: F401
        return
    except ImportError:
        pass
    try:
        from trn_agent_boot.trn_boot import _ntff_profile_via_ctypes
        hook = _ntff_profile_via_ctypes("/opt/axon/libaxon_pjrt.so")
        if hook is None:
            return
        import antenv
        mod = types.ModuleType("antenv.axon_hooks")
        mod.get_axon_ntff_profile_hook = lambda: hook
        mod.set_axon_ntff_profile_hook = lambda h: None
        sys.modules["antenv.axon_hooks"] = mod
        antenv.axon_hooks = mod
    except Exception:
        pass


# ---------------------------------------------------------------- device IR
def build_module(world=8, tok_shard=None):
    import concourse.bacc as bacc
    import concourse.mybir as mybir
    import concourse.tile as tile
    from concourse.masks import make_identity

    fp32 = mybir.dt.float32
    bf16 = mybir.dt.bfloat16
    AF = mybir.ActivationFunctionType
    ALU = mybir.AluOpType

    if tok_shard is None:
        tok_shard = T // world

    nc = bacc.Bacc(None, target_bir_lowering=False)

    # ---- I/O
    xT = nc.declare_dram_parameter("xT", [D, tok_shard], bf16, isOutput=False)
    wproj = nc.declare_dram_parameter("wproj", [D, PROJ_COLS], bf16, isOutput=False)
    convw = nc.declare_dram_parameter("convw", [6 * 128, KCONV], fp32, isOutput=False)
    dtb_col = nc.declare_dram_parameter("dtb_col", [128, 1], fp32, isOutput=False)
    negA_col = nc.declare_dram_parameter("negA_col", [128, 1], fp32, isOutput=False)
    normw_col = nc.declare_dram_parameter("normw_col", [128, 1], fp32, isOutput=False)
    wout = nc.declare_dram_parameter("wout", [world * 256, 256], bf16, isOutput=False)
    out_colsT = nc.declare_dram_parameter("out_colsT", [256, T], fp32, isOutput=True)

    # ---- internal DRAM
    HD = D // 2
    ag_in0 = nc.dram_tensor("ag_in0", [HD, tok_shard], bf16)
    ag_in1 = nc.dram_tensor("ag_in1", [HD, tok_shard], bf16)
    xT_h0 = nc.dram_tensor("xT_h0", [world * HD, tok_shard], bf16,
                           addr_space="Shared")
    xT_h1 = nc.dram_tensor("xT_h1", [world * HD, tok_shard], bf16,
                           addr_space="Shared")
    qkvT = nc.dram_tensor("qkvT", [6 * 128, T], bf16)    # q0 q1 k0 k1 v0 v1
    zT = nc.dram_tensor("zT", [2 * 128, T], bf16)        # z0 z1 (silu * norm_w)
    rows_dram = nc.dram_tensor("rows_dram", [8, T], fp32)  # b0 b1 a0 a1 sq0 sq1 sk0 sk1
    scal_dram = nc.dram_tensor("scal_dram", [9, 128, 128], fp32)
    O_dram = nc.dram_tensor("O_dram", [T, 2 * 128], bf16)   # pre-gate core, token-major
    rms_dram = nc.dram_tensor("rms_dram", [128, 128], fp32)  # [rp, t]
    NSEG = 4
    TSEG = T // NSEG
    coreT_s = [nc.dram_tensor(f"coreT{s}", [2 * 128, TSEG], bf16)
               for s in range(NSEG)]
    cT_all_s = [nc.dram_tensor(f"cT_all{s}", [world * 256, TSEG], bf16,
                               addr_space="Shared")
                for s in range(NSEG)]

    if DEBUG_TAPS:
        dbg_qkvT = nc.declare_dram_parameter("dbg_qkvT", [6 * 128, T], fp32, isOutput=True)
        dbg_zT = nc.declare_dram_parameter("dbg_zT", [2 * 128, T], fp32, isOutput=True)
        dbg_rows = nc.declare_dram_parameter("dbg_rows", [8, T], fp32, isOutput=True)
        dbg_coreT = nc.declare_dram_parameter("dbg_coreT", [2 * 128, T], fp32, isOutput=True)
        dbg_chunk = nc.declare_dram_parameter("dbg_chunk", [10, 128, 128], fp32, isOutput=True)

    groups = [list(range(world))]

    from contextlib import ExitStack
    with tile.TileContext(nc) as tc, ExitStack() as stack:
        # ============================================================ consts
        const = stack.enter_context(tc.tile_pool(name="const", bufs=1))
        ident_bf = const.tile([128, 128], bf16)
        make_identity(nc, ident_bf)
        ones128_bf = const.tile([128, 1], bf16)
        nc.vector.memset(ones128_bf[:], 1.0)
        ones1_f = const.tile([1, 128], fp32)
        nc.vector.memset(ones1_f[:], 1.0)
        eye_bf = const.tile([128, 128], bf16)
        make_identity(nc, eye_bf)
        maskS = const.tile([128, 128], fp32)   # 0 where j<t else NEG
        nc.gpsimd.memset(maskS[:], NEG)
        nc.gpsimd.affine_select(out=maskS[:], in_=maskS[:],
                                compare_op=ALU.is_ge, fill=0.0, base=0,
                                pattern=[[-1, 128]], channel_multiplier=1)
        maskI = const.tile([128, 128], fp32)   # -0.5*ln(DK) where i<=t else NEG
        nc.gpsimd.memset(maskI[:], NEG)
        nc.gpsimd.affine_select(out=maskI[:], in_=maskI[:],
                                compare_op=ALU.is_gt, fill=-HALF_LN_DK, base=0,
                                pattern=[[-1, 128]], channel_multiplier=1)
        # 6*128 channels -> [128, 6, KCONV]
        convw_sb = const.tile([128, 6, KCONV], fp32)
        nc.sync.dma_start(out=convw_sb[:],
                          in_=convw.ap().rearrange("(g p) k -> p g k", p=128))
        dtb_sb = const.tile([128, 1], fp32)
        nc.sync.dma_start(out=dtb_sb[:], in_=dtb_col[:])
        negA_sb = const.tile([128, 1], fp32)
        nc.sync.dma_start(out=negA_sb[:], in_=negA_col[:])
        normw_sb = const.tile([128, 1], fp32)
        nc.sync.dma_start(out=normw_sb[:], in_=normw_col[:])
        eps_col = const.tile([128, 1], fp32)
        nc.vector.memset(eps_col[:], EPS)
        nhldk_col = const.tile([128, 1], fp32)
        nc.vector.memset(nhldk_col[:], -HALF_LN_DK)

        # ============================================================ phase 1
        # AllGather X^T in two D-halves so first-half matmuls overlap AG #2
        nc.sync.dma_start(out=ag_in0[:], in_=xT[0:HD, :])
        nc.sync.dma_start(out=ag_in1[:], in_=xT[HD:D, :])
        nc.gpsimd.collective_compute(
            "AllGather", ALU.bypass, replica_groups=groups,
            ins=[ag_in0.ap().opt()], outs=[xT_h0.ap().opt()])
        nc.gpsimd.collective_compute(
            "AllGather", ALU.bypass, replica_groups=groups,
            ins=[ag_in1.ap().opt()], outs=[xT_h1.ap().opt()])

        KCH = KC // 2
        xT_view0 = xT_h0.ap().rearrange("(w kc p) t -> w kc p t", w=world, p=128)
        xT_view1 = xT_h1.ap().rearrange("(w kc p) t -> w kc p t", w=world, p=128)

        with (
            tc.tile_pool(name="pw", bufs=1) as pw,
            tc.tile_pool(name="px", bufs=2) as px,
            tc.tile_pool(name="pev", bufs=2) as pev,
            tc.tile_pool(name="phal", bufs=1) as phal,
            tc.tile_pool(name="pps", bufs=1, space="PSUM") as pps,
            tc.tile_pool(name="pss", bufs=2, space="PSUM") as pss,
        ):
            wsb = pw.tile([128, KC, PROJ_COLS], bf16)
            nc.sync.dma_start(out=wsb[:],
                              in_=wproj.ap().rearrange("(kc p) m -> p kc m", p=128))

            halos = [phal.tile([128, KCONV - 1], fp32, name=f"halo{g}") for g in range(6)]
            # rows collected on-chip: b0 b1 a0 a1 | ssq q0 q1 k0 k1


            GW = 4 * TT_W                # token-group width (4 psum banks)
            for ttg in range(NTT // 4):
                xtile = px.tile([128, KC, GW], bf16, tag="xt", name="xt")
                for i in range(4):
                    tt = ttg * 4 + i
                    g0 = tt * TT_W
                    blk = g0 // tok_shard
                    off = g0 % tok_shard
                    nc.sync.dma_start(out=xtile[:, 0:KCH, i * TT_W:(i + 1) * TT_W],
                                      in_=xT_view0[blk, :, :, off:off + TT_W]
                                      .transpose([1, 0, 2]))
                    nc.sync.dma_start(out=xtile[:, KCH:KC, i * TT_W:(i + 1) * TT_W],
                                      in_=xT_view1[blk, :, :, off:off + TT_W]
                                      .transpose([1, 0, 2]))
                for m in range(MCOL):
                    psums = [pps.tile([128, TT_W], fp32, tag=f"pp{i}",
                                      name=f"pp{i}") for i in range(4)]
                    for kc in range(KC):
                        for i in range(4):
                            nc.tensor.matmul(psums[i][:],
                                             wsb[:, kc, m * 128:(m + 1) * 128],
                                             xtile[:, kc, i * TT_W:(i + 1) * TT_W],
                                             start=(kc == 0), stop=(kc == KC - 1))
                    for i in range(4):
                        tt = ttg * 4 + i
                        g0 = tt * TT_W
                        psum = psums[i]
                        batch_start = (g0 % S) == 0
                        if m < 6:
                            # causal conv + silu -> qkvT
                            xe = pev.tile([128, TT_W + KCONV - 1], fp32, tag="xe")
                            if batch_start:
                                nc.vector.memset(xe[:, 0:KCONV - 1], 0.0)
                            else:
                                nc.vector.tensor_copy(xe[:, 0:KCONV - 1], halos[m][:])
                            nc.scalar.copy(xe[:, KCONV - 1:], psum[:])
                            nc.vector.tensor_copy(halos[m][:], psum[:, TT_W - (KCONV - 1):])
                            wcol = convw_sb[:, m, :]
                            y0 = pev.tile([128, TT_W], fp32, tag="y0")
                            y1 = pev.tile([128, TT_W], fp32, tag="y1")
                            nc.vector.tensor_scalar_mul(y0[:], xe[:, 3:3 + TT_W],
                                                        wcol[:, 3:4])
                            nc.vector.scalar_tensor_tensor(
                                y1[:], xe[:, 2:2 + TT_W], wcol[:, 2:3], y0[:],
                                op0=ALU.mult, op1=ALU.add)
                            nc.vector.scalar_tensor_tensor(
                                y0[:], xe[:, 1:1 + TT_W], wcol[:, 1:2], y1[:],
                                op0=ALU.mult, op1=ALU.add)
                            nc.vector.scalar_tensor_tensor(
                                y1[:], xe[:, 0:0 + TT_W], wcol[:, 0:1], y0[:],
                                op0=ALU.mult, op1=ALU.add)
                            sg = pev.tile([128, TT_W], fp32, tag="sg")
                            nc.scalar.activation(sg[:], y1[:], AF.Sigmoid)
                            act = pev.tile([128, TT_W], bf16, tag="act")
                            nc.vector.tensor_tensor(act[:], y1[:], sg[:], op=ALU.mult)
                            nc.sync.dma_start(out=qkvT[m * 128:(m + 1) * 128,
                                                       g0:g0 + TT_W], in_=act[:])
                            if m < 4:
                                sq = pev.tile([128, TT_W], bf16, tag="sq")
                                nc.scalar.activation(sq[:], act[:], AF.Square)
                                ss_ps = pss.tile([1, TT_W], fp32)
                                nc.tensor.matmul(ss_ps[:], ones128_bf[:], sq[:],
                                                 start=True, stop=True)
                                ssst = pev.tile([1, TT_W], fp32, tag="ssst")
                                nc.vector.tensor_copy(ssst[:], ss_ps[:])
                                nc.sync.dma_start(
                                    out=rows_dram[4 + m:5 + m, g0:g0 + TT_W],
                                    in_=ssst[:])
                        elif m < 8:
                            # z: silu * norm_w
                            zs = pev.tile([128, TT_W], fp32, tag="zs")
                            nc.scalar.activation(zs[:], psum[:], AF.Sigmoid)
                            zb = pev.tile([128, TT_W], bf16, tag="zb")
                            nc.vector.scalar_tensor_tensor(
                                zb[:], psum[:], normw_sb[:], zs[:],
                                op0=ALU.mult, op1=ALU.mult)
                            nc.sync.dma_start(out=zT[(m - 6) * 128:(m - 5) * 128,
                                                     g0:g0 + TT_W], in_=zb[:])
                        else:
                            bast = pev.tile([4, TT_W], fp32, tag="bast")
                            nc.vector.tensor_copy(bast[:], psum[0:4, :])
                            nc.sync.dma_start(out=rows_dram[0:4, g0:g0 + TT_W],
                                              in_=bast[:])



        if DEBUG_TAPS:
            with tc.tile_pool(name="dbg", bufs=2) as pd:
                for g in range(6):
                    for j in range(4):
                        tdb = pd.tile([128, 2048], bf16, tag="tdb")
                        nc.sync.dma_start(out=tdb[:], in_=qkvT[g * 128:(g + 1) * 128, j * 2048:(j + 1) * 2048])
                        tdf = pd.tile([128, 2048], fp32, tag="tdf")
                        nc.vector.tensor_copy(tdf[:], tdb[:])
                        nc.sync.dma_start(out=dbg_qkvT[g * 128:(g + 1) * 128, j * 2048:(j + 1) * 2048], in_=tdf[:])
                for g in range(2):
                    for j in range(4):
                        tdb = pd.tile([128, 2048], bf16, tag="tdb")
                        nc.sync.dma_start(out=tdb[:], in_=zT[g * 128:(g + 1) * 128, j * 2048:(j + 1) * 2048])
                        tdf = pd.tile([128, 2048], fp32, tag="tdf")
                        nc.vector.tensor_copy(tdf[:], tdb[:])
                        nc.sync.dma_start(out=dbg_zT[g * 128:(g + 1) * 128, j * 2048:(j + 1) * 2048], in_=tdf[:])
                for j in range(8):
                    tdf = pd.tile([8, 1024], fp32, tag="tdr")
                    nc.sync.dma_start(out=tdf[:], in_=rows_dram[:, j * 1024:(j + 1) * 1024])
                    nc.sync.dma_start(out=dbg_rows[:, j * 1024:(j + 1) * 1024], in_=tdf[:])

        # ====================================================== phase 1.5
        # Per-token scalar machinery. Big tiles [128 = (lh,b,chunk), 128 = t].
        with (
            tc.tile_pool(name="pscal", bufs=1) as ps,
            tc.tile_pool(name="pscal2", bufs=2) as ps2,
        ):
            def big_from_rows(row0, name):
                t_ = ps.tile([128, 128], fp32, name=name)
                nc.sync.dma_start(
                    out=t_[:],
                    in_=rows_dram.ap()[row0:row0 + 2, :]
                    .rearrange("lh (b n t) -> (lh b n) t", b=B, n=NCHUNK, t=C))
                return t_

            blog_big = big_from_rows(0, "blog")
            alog_big = big_from_rows(2, "alog")
            ssqq_big = big_from_rows(4, "ssqq")
            ssqk_big = big_from_rows(6, "ssqk")

            zeros_big = ps.tile([128, 128], fp32, name="zeros")
            nc.vector.memset(zeros_big[:], 0.0)

            # softplus(x) = relu(x) + ln(1 + exp(-|x|)), x = alog + dtb
            xsp = ps.tile([128, 128], fp32, name="xsp")
            nc.vector.tensor_scalar_add(xsp[:], alog_big[:], dtb_sb[:])
            t_abs = ps.tile([128, 128], fp32, name="t_abs")
            nc.scalar.activation(t_abs[:], xsp[:], AF.Abs)
            nc.scalar.activation(t_abs[:], t_abs[:], AF.Exp, scale=-1.0)
            nc.scalar.activation(t_abs[:], t_abs[:], AF.Ln, bias=1.0)
            t_rel = ps.tile([128, 128], fp32, name="t_rel")
            nc.scalar.activation(t_rel[:], xsp[:], AF.Relu)
            g_big = ps.tile([128, 128], fp32, name="g")
            nc.vector.tensor_tensor(g_big[:], t_abs[:], t_rel[:], op=ALU.add)
            nc.vector.tensor_scalar_mul(g_big[:], g_big[:], negA_sb[:])
            gam_big = ps.tile([128, 128], fp32, name="gam")
            nc.vector.tensor_tensor_scan(gam_big[:], g_big[:], zeros_big[:], 0.0,
                                         op0=ALU.add, op1=ALU.add)

            beta_big = ps.tile([128, 128], fp32, name="beta")
            nc.scalar.activation(beta_big[:], blog_big[:], AF.Exp, scale=-1.0)
            nc.vector.tensor_scalar_add(beta_big[:], beta_big[:], 1.0)
            nc.vector.reciprocal(beta_big[:], beta_big[:])
            nbeta_big = ps.tile([128, 128], fp32, name="nbeta")
            nc.scalar.mul(nbeta_big[:], beta_big[:], -1.0)

            lnq = ps2.tile([128, 128], fp32, tag="ln")
            nc.scalar.activation(lnq[:], ssqq_big[:], AF.Ln, bias=eps_col[:])
            u_q_big = ps.tile([128, 128], fp32, name="u_q")
            nc.vector.scalar_tensor_tensor(u_q_big[:], lnq[:], -0.5, gam_big[:],
                                           op0=ALU.mult, op1=ALU.add)
            qbar_big = ps.tile([128, 128], fp32, name="qbar")
            nc.scalar.activation(qbar_big[:], u_q_big[:], AF.Exp, bias=nhldk_col[:])

            lnk = ps2.tile([128, 128], fp32, tag="ln")
            nc.scalar.activation(lnk[:], ssqk_big[:], AF.Ln, bias=eps_col[:])
            u_k_big = ps.tile([128, 128], fp32, name="u_k")
            nc.vector.scalar_tensor_tensor(u_k_big[:], lnk[:], -0.5, gam_big[:],
                                           op0=ALU.mult, op1=ALU.add)
            v_k_big = ps.tile([128, 128], fp32, name="v_k")
            nc.vector.scalar_tensor_tensor(v_k_big[:], lnk[:], 0.5, gam_big[:],
                                           op0=ALU.mult, op1=ALU.add)
            s1_big = ps.tile([128, 128], fp32, name="s1")
            nc.scalar.activation(s1_big[:], u_k_big[:], AF.Exp)

            gl_col = gam_big[:, 127:128]
            nv_big = ps2.tile([128, 128], fp32, tag="nv")
            nc.scalar.mul(nv_big[:], v_k_big[:], -1.0)
            s2_big = ps.tile([128, 128], fp32, name="s2")
            nc.scalar.activation(s2_big[:], nv_big[:], AF.Exp, bias=gl_col)
            dl_col = ps.tile([128, 1], fp32, name="dl")
            nc.scalar.activation(dl_col[:], gl_col, AF.Exp)

            # column layouts via DRAM bounce: scal_dram[q] = big[rp, t]
            for q, t_ in enumerate([v_k_big, beta_big, nbeta_big, s1_big, s2_big,
                                    None, u_k_big, u_q_big, qbar_big]):
                if t_ is not None:
                    nc.sync.dma_start(out=scal_dram[q], in_=t_[:])
            nc.sync.dma_start(out=scal_dram[5][:, 0:1], in_=dl_col[:])

            cols_vk = ps.tile([128, 128], fp32, name="cvk")
            cols_beta = ps.tile([128, 128], fp32, name="cbe")
            cols_nbeta = ps.tile([128, 128], fp32, name="cnb")
            cols_s1 = ps.tile([128, 128], fp32, name="cs1")
            cols_s2 = ps.tile([128, 128], fp32, name="cs2")
            for q, t_ in enumerate([cols_vk, cols_beta, cols_nbeta, cols_s1, cols_s2]):
                nc.sync.dma_start(out=t_[:],
                                  in_=scal_dram[q].rearrange("rp t -> t rp"))
            dl_row = ps.tile([1, 128], fp32, name="dlrow")
            nc.sync.dma_start(out=dl_row[:],
                              in_=scal_dram[5][:, 0:1].rearrange("rp one -> one rp"))
            dec_bc = ps.tile([128, 128], fp32, name="dec")
            nc.gpsimd.partition_broadcast(dec_bc[:], dl_row[:])

            # ====================================================== phase 2
            # chunked gated delta rule
            with (
                tc.tile_pool(name="rin", bufs=6) as rin,
                tc.tile_pool(name="rwk", bufs=4) as rwk,
                tc.tile_pool(name="rst", bufs=1) as rst,
                tc.tile_pool(name="ow", bufs=1) as ow,
                tc.tile_pool(name="oc", bufs=1) as oc,
                tc.tile_pool(name="oe", bufs=4) as oe,
                tc.tile_pool(name="rgp", bufs=2, space="PSUM") as rgp,
                tc.tile_pool(name="rps", bufs=2, space="PSUM") as rps,
                tc.tile_pool(name="ops", bufs=1, space="PSUM") as ops,
            ):
                wout_sb = ow.tile([128, 2 * world, 256], bf16)
                nc.sync.dma_start(out=wout_sb[:],
                                  in_=wout.ap().rearrange("(wk p) m -> p wk m",
                                                          p=128))
                ssq_cols = rst.tile([128, 128], fp32, name="ssqc")
                S_f32 = [rst.tile([128, 128], fp32, name=f"Sf{p}") for p in range(NPAIR)]
                S_bf = [rst.tile([128, 128], bf16, name=f"Sbf{p}") for p in range(NPAIR)]
                for p in range(NPAIR):
                    nc.vector.memset(S_f32[p][:], 0.0)
                    nc.vector.memset(S_bf[p][:], 0.0)

                for n in range(NCHUNK):
                    for p in range(NPAIR):
                        lh, b = p >> 1, p & 1
                        rp = 32 * p + n
                        tok0 = b * S + n * C

                        kq = rin.tile([128, 256], bf16, tag="kq")
                        kT = kq[:, 0:128]
                        qT = kq[:, 128:256]
                        vT = rin.tile([128, 128], bf16, tag="vT")
                        nc.sync.dma_start(out=kq[:, 0:128], in_=qkvT[(2 + lh) * 128:(3 + lh) * 128, tok0:tok0 + C])
                        nc.sync.dma_start(out=kq[:, 128:256], in_=qkvT[lh * 128:(lh + 1) * 128, tok0:tok0 + C])
                        nc.sync.dma_start(out=vT[:], in_=qkvT[(4 + lh) * 128:(5 + lh) * 128, tok0:tok0 + C])

                        # broadcast rows u_k,u_q,qbar: tiny row DMAs to
                        # partition 0, replicate on the (idle) gpsimd engine
                        bc_st = rwk.tile([1, 384], fp32, tag="bcst", name="bcst")
                        for qi in range(3):
                            nc.sync.dma_start(
                                out=bc_st[:, qi * 128:(qi + 1) * 128],
                                in_=scal_dram[6 + qi][rp:rp + 1, :])
                        bc_ps = rwk.tile([128, 384], fp32, tag="bc", name="bc")
                        nc.gpsimd.partition_broadcast(bc_ps[:], bc_st[:])

                        # [G | P] = K [K | Q]^T in one matmul (raw, [j|i, t])
                        GP_ps = rgp.tile([128, 256], fp32, tag="gp")
                        nc.tensor.matmul(GP_ps[:], kT, kq[:], start=True, stop=True)
                        G_ps = GP_ps[:, 0:128]
                        P_ps = GP_ps[:, 128:256]

                        # N = -M2^T  (strictly upper in [j,t])
                        E1 = rwk.tile([128, 128], fp32, tag="E1")
                        nc.vector.scalar_tensor_tensor(
                            E1[:], bc_ps[:, 0:128], cols_vk[:, rp:rp + 1], maskS[:],
                            op0=ALU.subtract, op1=ALU.add)
                        nc.scalar.activation(E1[:], E1[:], AF.Exp)
                        N0 = rwk.tile([128, 128], bf16, tag="N0")
                        nc.vector.scalar_tensor_tensor(
                            N0[:], G_ps, cols_nbeta[:, rp:rp + 1], E1[:],
                            op0=ALU.mult, op1=ALU.mult)

                        # squaring chain: S_j = N^(2^j), T_j = S_j^T
                        # X = Tinv2^T ~= (I+N2)(I+N) = I+N+N2+N3 (N^4 ~ 1e-4,
                        # below bf16 noise: powers of the delta-rule matrix decay
                        # fast under the gamma decay + l2-normalized keys)
                        T0_ps = rps.tile([128, 128], bf16, tag="pst")
                        nc.tensor.transpose(T0_ps[:], N0[:], ident_bf[:])
                        T0 = rwk.tile([128, 128], bf16, tag="T0", name="T0")
                        nc.scalar.copy(T0[:], T0_ps[:])
                        t1_ps = rps.tile([128, 128], fp32, tag="ps")
                        nc.tensor.matmul(t1_ps[:], N0[:], T0[:], start=True, stop=True)
                        T1 = rwk.tile([128, 128], bf16, tag="T1", name="T1")
                        nc.scalar.copy(T1[:], t1_ps[:])
                        X0 = rwk.tile([128, 128], bf16, tag="X0", name="X0")
                        nc.vector.tensor_tensor(X0[:], eye_bf[:], N0[:], op=ALU.add)
                        xp = rps.tile([128, 128], fp32, tag="ps")
                        nc.tensor.matmul(xp[:], T1[:], X0[:], start=True, stop=True)
                        X = rwk.tile([128, 128], bf16, tag="X")
                        nc.vector.tensor_tensor(X[:], X0[:], xp[:], op=ALU.add)

                        # token-major V, K variants
                        vt_ps = rps.tile([128, 128], bf16, tag="pst")
                        nc.tensor.transpose(vt_ps[:], vT[:], ident_bf[:])
                        V_tok = rwk.tile([128, 128], bf16, tag="Vtok")
                        nc.vector.tensor_copy(V_tok[:], vt_ps[:])
                        kt_ps = rps.tile([128, 128], bf16, tag="pst")
                        nc.tensor.transpose(kt_ps[:], kT[:], ident_bf[:])
                        KG = rwk.tile([128, 128], bf16, tag="KG")
                        nc.scalar.activation(KG[:], kt_ps[:], AF.Copy,
                                             scale=cols_s1[:, rp:rp + 1])
                        Kd = rwk.tile([128, 128], bf16, tag="Kd")
                        nc.vector.tensor_scalar_mul(Kd[:], kt_ps[:],
                                                    cols_s2[:, rp:rp + 1])

                        # W2^T (negated)
                        wt_ps = rps.tile([128, 128], fp32, tag="ps")
                        nc.tensor.matmul(wt_ps[:], KG[:], X[:], start=True, stop=True)
                        nWt = rwk.tile([128, 128], bf16, tag="nWt")
                        nc.scalar.mul(nWt[:], wt_ps[:], -1.0)

                        # R = beta * (U2 - W2 S)
                        R_ps = rps.tile([128, 128], fp32, tag="ps")
                        nc.tensor.matmul(R_ps[:], X[:], V_tok[:], start=True, stop=False)
                        nc.tensor.matmul(R_ps[:], nWt[:], S_bf[p][:], start=False, stop=True)
                        R = rwk.tile([128, 128], bf16, tag="R")
                        nc.scalar.activation(R[:], R_ps[:], AF.Copy,
                                             scale=cols_beta[:, rp:rp + 1])

                        # DA^T = P * exp(E) masked incl diag
                        E3 = rwk.tile([128, 128], fp32, tag="E3")
                        nc.vector.scalar_tensor_tensor(
                            E3[:], bc_ps[:, 128:256], cols_vk[:, rp:rp + 1], maskI[:],
                            op0=ALU.subtract, op1=ALU.add)
                        nc.scalar.activation(E3[:], E3[:], AF.Exp)
                        DA = rwk.tile([128, 128], bf16, tag="DA")
                        nc.vector.tensor_tensor(DA[:], P_ps, E3[:], op=ALU.mult)

                        # qbar^T = qT * bcast(qbar)
                        qbT = rwk.tile([128, 128], bf16, tag="qbT")
                        nc.vector.tensor_tensor(qbT[:], qT[:], bc_ps[:, 256:384],
                                                op=ALU.mult)

                        # O = qbar^T' S + DA' R  -> [t, dv]
                        O_ps = rps.tile([128, 128], fp32, tag="ps")
                        nc.tensor.matmul(O_ps[:], qbT[:], S_bf[p][:], start=True, stop=False)
                        nc.tensor.matmul(O_ps[:], DA[:], R[:], start=False, stop=True)

                        if DEBUG_TAPS and rp == 64:
                            for di, dt_ in enumerate([("N0", N0), ("X", X), ("KG", KG),
                                                      ("Kd", Kd), ("R", R), ("DA", DA),
                                                      ("qbT", qbT), ("Vtok", V_tok)]):
                                dnm, dtile = dt_
                                dcp = rwk.tile([128, 128], fp32, tag="dcp", name=f"dcp{di}")
                                nc.vector.tensor_copy(dcp[:], dtile[:])
                                nc.sync.dma_start(out=dbg_chunk[di], in_=dcp[:])
                            dcp8 = rwk.tile([128, 128], fp32, tag="dcp", name="dcp8")
                            nc.vector.tensor_copy(dcp8[:], O_ps[:])
                            nc.sync.dma_start(out=dbg_chunk[8], in_=dcp8[:])
                            dcp9 = rwk.tile([128, 128], fp32, tag="dcp", name="dcp9")
                            nc.vector.tensor_copy(dcp9[:], E1[:])
                            nc.sync.dma_start(out=dbg_chunk[9], in_=dcp9[:])

                        # state update: S = dec * S + Kd^T R
                        up_ps = rps.tile([128, 128], fp32, tag="ps")
                        nc.tensor.matmul(up_ps[:], Kd[:], R[:], start=True, stop=True)
                        nc.vector.scalar_tensor_tensor(
                            S_f32[p][:], S_f32[p][:], dec_bc[:, rp:rp + 1], up_ps[:],
                            op0=ALU.mult, op1=ALU.add)
                        nc.vector.tensor_copy(S_bf[p][:], S_f32[p][:])

                        # stash O (token-major) + row sum-of-squares; the
                        # rms + gating + transpose run in a later batched pass
                        sqs = rwk.tile([128, 128], fp32, tag="sqs")
                        nc.scalar.activation(sqs[:], O_ps[:], AF.Square,
                                             accum_out=ssq_cols[:, rp:rp + 1])
                        O_sb = rwk.tile([128, 128], bf16, tag="Osb")
                        nc.vector.tensor_copy(O_sb[:], O_ps[:])
                        nc.sync.dma_start(
                            out=O_dram[tok0:tok0 + C, lh * 128:(lh + 1) * 128],
                            in_=O_sb[:])

                # batched rms: ssq_cols [t, rp] -> [rp, t] -> exp(-0.5 ln(mean+eps))
                nc.sync.dma_start(out=scal_dram[5], in_=ssq_cols[:])
                ssq_rp = rst.tile([128, 128], fp32, name="ssqrp")
                nc.sync.dma_start(out=ssq_rp[:],
                                  in_=scal_dram[5].rearrange("t rp -> rp t"))
                nc.scalar.activation(ssq_rp[:], ssq_rp[:], AF.Ln, scale=1.0 / DV,
                                     bias=eps_col[:])
                nc.scalar.activation(ssq_rp[:], ssq_rp[:], AF.Exp, scale=-0.5)
                nc.sync.dma_start(out=rms_dram[:], in_=ssq_rp[:])

                # phase B: gate + transpose, segment by token range; AG per segment
                for s in range(NSEG):
                    b_s = s // 2
                    for n_s in range((s % 2) * 16, (s % 2) * 16 + 16):
                        for lh_s in range(2):
                            p_s = lh_s * 2 + b_s
                            rp_s = 32 * p_s + n_s
                            tok0s = b_s * S + n_s * C
                            Oin = rwk.tile([128, 128], bf16, tag="Oin", name="Oin")
                            nc.sync.dma_start(
                                out=Oin[:],
                                in_=O_dram[tok0s:tok0s + C,
                                           lh_s * 128:(lh_s + 1) * 128])
                            zin = rwk.tile([128, 128], bf16, tag="zin", name="zin")
                            nc.sync.dma_start(
                                out=zin[:],
                                in_=zT[lh_s * 128:(lh_s + 1) * 128,
                                       tok0s:tok0s + C])
                            rms_st = rwk.tile([1, 128], fp32, tag="rmsst",
                                              name="rmsst")
                            nc.sync.dma_start(out=rms_st[:],
                                              in_=rms_dram[rp_s:rp_s + 1, :])
                            rmsb = rwk.tile([128, 128], fp32, tag="rmsb",
                                            name="rmsb")
                            nc.gpsimd.partition_broadcast(rmsb[:], rms_st[:])
                            ot_ps = rps.tile([128, 128], bf16, tag="pst")
                            nc.tensor.transpose(ot_ps[:], Oin[:], ident_bf[:])
                            gr = rwk.tile([128, 128], fp32, tag="gr", name="gr")
                            nc.vector.tensor_tensor(gr[:], ot_ps[:], rmsb[:],
                                                    op=ALU.mult)
                            gct = rwk.tile([128, 128], bf16, tag="gct", name="gct")
                            nc.vector.tensor_tensor(gct[:], gr[:], zin[:],
                                                    op=ALU.mult)
                            nc.sync.dma_start(
                                out=coreT_s[s][lh_s * 128:(lh_s + 1) * 128,
                                               tok0s - s * TSEG:
                                               tok0s - s * TSEG + C],
                                in_=gct[:])
                    nc.gpsimd.collective_compute(
                        "AllGather", ALU.bypass, replica_groups=groups,
                        ins=[coreT_s[s].ap().opt()], outs=[cT_all_s[s].ap().opt()])

                    # out_proj for this segment (overlaps next segment's gating)
                    cview = cT_all_s[s].ap().rearrange("(wk p) t -> wk p t",
                                                       p=128)
                    for sub in range(2):
                        ctiles = []
                        for i in range(2):
                            ctile = oc.tile([128, 2 * world, TT_W], bf16,
                                            tag=f"ct{i}", name=f"ct{i}")
                            toff = (sub * 2 + i) * TT_W
                            nc.sync.dma_start(
                                out=ctile[:],
                                in_=cview[:, :, toff:toff + TT_W]
                                .transpose([1, 0, 2]))
                            ctiles.append(ctile)
                        for och in range(2):
                            psums = [ops.tile([128, TT_W], fp32, tag=f"po{i}",
                                              name=f"po{i}") for i in range(2)]
                            for wk in range(2 * world):
                                for i in range(2):
                                    nc.tensor.matmul(
                                        psums[i][:],
                                        wout_sb[:, wk, och * 128:(och + 1) * 128],
                                        ctiles[i][:, wk, :],
                                        start=(wk == 0), stop=(wk == 2 * world - 1))
                            for i in range(2):
                                oev = oe.tile([128, TT_W], fp32, tag="oev",
                                              name="oev")
                                nc.scalar.copy(oev[:], psums[i][:])
                                toff = (sub * 2 + i) * TT_W
                                nc.sync.dma_start(
                                    out=out_colsT[och * 128:(och + 1) * 128,
                                                  s * TSEG + toff:
                                                  s * TSEG + toff + TT_W],
                                    in_=oev[:])

        if DEBUG_TAPS:
            with tc.tile_pool(name="dbg2", bufs=2) as pd:
                for g in range(2):
                    for j in range(4):
                        tdb = pd.tile([128, 2048], bf16, tag="tdb2")
                        nc.sync.dma_start(out=tdb[:], in_=coreT_s[j][g * 128:(g + 1) * 128, :])
                        tdf = pd.tile([128, 2048], fp32, tag="tdf2")
                        nc.vector.tensor_copy(tdf[:], tdb[:])
                        nc.sync.dma_start(out=dbg_coreT[g * 128:(g + 1) * 128, j * 2048:(j + 1) * 2048], in_=tdf[:])

    nc.finalize()
    return nc


# ---------------------------------------------------------------- host side
def _to_bf16(a):
    import ml_dtypes
    return np.asarray(a, dtype=np.float32).astype(ml_dtypes.bfloat16)


def _build_in_maps(inputs, world=8, tok_shard=None):
    if tok_shard is None:
        tok_shard = T // world
    hs = np.ascontiguousarray(np.asarray(inputs["hidden_states"],
                                         dtype=np.float32).reshape(T, D))
    W_qkv = np.asarray(inputs["W_qkv"], dtype=np.float32)
    W_z = np.asarray(inputs["W_z"], dtype=np.float32)
    W_b = np.asarray(inputs["W_b"], dtype=np.float32)
    W_a = np.asarray(inputs["W_a"], dtype=np.float32)
    conv_w = np.asarray(inputs["conv_w"], dtype=np.float32)[:, 0, :]  # [CONV_DIM, 4]
    A_log = np.asarray(inputs["A_log"], dtype=np.float32)
    dt_bias = np.asarray(inputs["dt_bias"], dtype=np.float32)
    norm_w = np.asarray(inputs["norm_w"], dtype=np.float32)
    W_out = np.asarray(inputs["W_out"], dtype=np.float32)

    in_maps = []
    for c in range(world):
        h0 = c * HL
        heads = [h0, h0 + 1]
        xT_shard = _to_bf16(hs[c * tok_shard:(c + 1) * tok_shard, :].T)

        wp = np.zeros((D, PROJ_COLS), np.float32)
        for i, h in enumerate(heads):
            wp[:, i * 128:(i + 1) * 128] = W_qkv[:, h * DK:(h + 1) * DK]
            wp[:, 256 + i * 128:256 + (i + 1) * 128] = W_qkv[:, KEY_DIM + h * DK:KEY_DIM + (h + 1) * DK]
            wp[:, 512 + i * 128:512 + (i + 1) * 128] = W_qkv[:, 2 * KEY_DIM + h * DV:2 * KEY_DIM + (h + 1) * DV]
            wp[:, 768 + i * 128:768 + (i + 1) * 128] = W_z[:, h * DV:(h + 1) * DV]
            wp[:, 1024 + i] = W_b[:, h]
            wp[:, 1026 + i] = W_a[:, h]

        cw = np.zeros((6 * 128, KCONV), np.float32)
        for i, h in enumerate(heads):
            cw[i * 128:(i + 1) * 128] = conv_w[h * DK:(h + 1) * DK]
            cw[(2 + i) * 128:(3 + i) * 128] = conv_w[KEY_DIM + h * DK:KEY_DIM + (h + 1) * DK]
            cw[(4 + i) * 128:(5 + i) * 128] = conv_w[2 * KEY_DIM + h * DV:2 * KEY_DIM + (h + 1) * DV]

        # rp = lh*64 + b*32 + n  ->  head = heads[rp // 64]
        lh_of_rp = np.arange(128) // 64
        dtb_col = dt_bias[np.array(heads)][lh_of_rp].reshape(128, 1)
        negA_col = (-np.exp(A_log))[np.array(heads)][lh_of_rp].reshape(128, 1)
        normw_col = norm_w.reshape(128, 1)
        wout_c = _to_bf16(W_out[:world * 256, c * 256:(c + 1) * 256])

        in_maps.append({
            "xT": xT_shard,
            "wproj": _to_bf16(wp),
            "convw": cw.astype(np.float32),
            "dtb_col": dtb_col.astype(np.float32),
            "negA_col": negA_col.astype(np.float32),
            "normw_col": normw_col.astype(np.float32),
            "wout": wout_c,
        })
    return in_maps


def kernel(hidden_states, W_qkv, W_z, W_b, W_a, conv_w, A_log, dt_bias,
           norm_w, W_out):
    global LAST_RESULT
    from concourse.bass_utils import run_bass_kernel_spmd

    _install_ntff_hook()
    if "nc" not in _CACHE:
        _CACHE["nc"] = build_module(world=8)
    nc = _CACHE["nc"]

    inputs = dict(hidden_states=hidden_states, W_qkv=W_qkv, W_z=W_z, W_b=W_b,
                  W_a=W_a, conv_w=conv_w, A_log=A_log, dt_bias=dt_bias,
                  norm_w=norm_w, W_out=W_out)
    in_maps = _build_in_maps(inputs, world=8)
    res = run_bass_kernel_spmd(nc, in_maps, core_ids=list(range(8)),
                               trace=bool(os.environ.get("BASS_TRACE")))
    LAST_RESULT = res
    out = np.concatenate([res.results[c]["out_colsT"].T for c in range(8)], axis=1)
    return out.reshape(B, S, D).astype(np.float32)
